# revision 18
# baseline (speedup 1.0000x reference)
"""Bipartite GNN message-passing kernel for 8 Trainium2 NeuronCores.

Strategy (self-contained; shapes hardcoded for the nn_BipartiteGNN problem):

  reference computes   relu(feat[idx] @ W)  per edge. Row-gather commutes with
  the matmul, so we project FIRST (dense, small) and gather 256-dim message
  rows instead of 1024-dim feature rows:

      u_msg = relu(dish_feat @ W_um)[edge_dish]     (same for d_msg)

  Sharding over 8 cores:
    * users:  50176 padded rows -> 392 blocks of 128 -> 49 blocks/core.
      Core c projects its user rows (user_init kept in SBUF, user_msg -> HBM)
      and aggregates/updates its own user blocks.
    * dishes: 10240 padded rows -> 80 blocks -> 10 blocks/core for
      projection/update.  dish_msg slices are AllGathered (5.2 MB bf16) so
      every core can gather any dish row locally.
    * dish-side aggregation is sender-sharded: core c scatter-adds messages of
      edges whose sender user lives on c into a full [10240,256] partial,
      then two aligned ReduceScatter chunks hand each core the summed
      aggregates for its own dish blocks.

  segment_sum is done as one-hot matmuls: per receiver block, gather the
  edge message rows (dma_gather, rows land one-per-partition) and multiply by
  host-built one-hot S tiles, accumulating in PSUM.  Dummy slots gather row 0
  (the negative-index trim path crashes this runtime) and carry S=0 columns;
  per-block tile counts are the max over the 8 cores, so the static SPMD
  program stays identical while padding stays small.

  The dish-scatter and user-block phases are interleaved so the TensorEngine
  always has dense matmul work while gathers stream in the background (HAM
  stays at 2.4 GHz).
"""

import os
import sys

import numpy as np
import ml_dtypes

if "/opt/trn_rl_repo" not in sys.path:
    sys.path.insert(0, "/opt/trn_rl_repo")

import concourse.bass as bass  # noqa: E402
import concourse.mybir as mybir  # noqa: E402
import concourse.tile as tile  # noqa: E402
from concourse import bacc  # noqa: E402
from concourse.bass_utils import run_bass_kernel_spmd  # noqa: E402

P = 128
NCORES = 8
N_USERS = 50000
N_DISHES = 10000
D_IN = 1024
HID = 256
OUT = 128

U_PAD = 50176            # 392 blocks of 128
D_PAD = 10240            # 80 blocks of 128
UBC = U_PAD // P // NCORES   # 49 user blocks per core
DBC = D_PAD // P // NCORES   # 10 dish blocks per core (update phase)
DB = D_PAD // P              # 80 dish blocks (scatter phase, all cores)
U_ROWS = UBC * P             # 6272 user rows per core
D_ROWS = DBC * P             # 1280 dish rows per core
RS_CHUNKS = 2                # dish_agg ReduceScatter split (640 rows/core each)

BF16 = mybir.dt.bfloat16
F32 = mybir.dt.float32
I16 = mybir.dt.int16

_cache = {}
NQ = int(os.environ.get("KNQ", "3"))


# --------------------------------------------------------------------------
# host-side prep
# --------------------------------------------------------------------------

def _wrap_idx_calls(idx_flat: np.ndarray) -> np.ndarray:
    """[n] int16 -> [128, n/16] wrapped gather-index layout: logical index i at
    partition i%16 (replicated across the 8 groups of 16), free slot i//16."""
    n = idx_flat.shape[0]
    assert n % 16 == 0
    w = idx_flat.reshape(n // 16, 16).T  # [16, n/16]
    return np.tile(w, (8, 1)).astype(np.int16)


def _build_side(recv_all, send_all, owner_of_edge, n_blocks, block_fn, send_base,
                block_perm=None):
    """For one aggregation side, build per-core gather-index + one-hot-S data
    plus the shared per-block tile counts T_b (max over cores, >=1).

    block_perm[i] = original block id processed at device step i; the idx/S
    layout follows the permuted order so device tile ranges stay contiguous.
    Returns (T_b [n_blocks] in permuted order, per_core list of
    (idx_wrapped, s_tiles)).
    """
    if block_perm is None:
        block_perm = np.arange(n_blocks)
    inv_perm = np.empty(n_blocks, np.int64)
    inv_perm[np.asarray(block_perm)] = np.arange(n_blocks)
    per_core = []
    counts_all = np.zeros((NCORES, n_blocks), np.int64)
    for c in range(NCORES):
        m = owner_of_edge == c
        send = send_all[m] - send_base[c]
        blk = inv_perm[block_fn(recv_all[m], c)]   # permuted block index
        order = np.argsort(blk, kind="stable")
        rb = blk[order]
        rl = (recv_all[m][order] % P).astype(np.int64)
        sl = send[order].astype(np.int64)
        counts = np.bincount(rb, minlength=n_blocks)
        counts_all[c] = counts
        per_core.append((counts, rb, rl, sl))
    T_b = np.maximum(np.ceil(counts_all.max(axis=0) / P).astype(np.int64), 1)
    offs = np.concatenate([[0], np.cumsum(T_b)])
    total_tiles = int(offs[-1])

    out = []
    for c in range(NCORES):
        counts, rb, rl, sl = per_core[c]
        starts = np.concatenate([[0], np.cumsum(counts)[:-1]])
        within = np.arange(len(rb), dtype=np.int64) - starts[rb]
        idx_flat = np.zeros((total_tiles * P,), dtype=np.int16)
        slot = offs[rb] * P + within
        idx_flat[slot] = sl.astype(np.int16)
        s = np.zeros((P, total_tiles * P), dtype=ml_dtypes.bfloat16)
        s[slot % P, (offs[rb] + within // P) * P + rl] = 1.0
        out.append((_wrap_idx_calls(idx_flat), s))
    return T_b, out


def _prep(inputs):
    """All numpy preprocessing; returns (in_maps, meta)."""
    user_feat = inputs["user_feat"]
    dish_feat = inputs["dish_feat"]
    edge_user = np.asarray(inputs["edge_user"], dtype=np.int64)
    edge_dish = np.asarray(inputs["edge_dish"], dtype=np.int64)

    uf = np.zeros((U_PAD, D_IN), np.float32)
    uf[:N_USERS] = user_feat
    df = np.zeros((D_PAD, D_IN), np.float32)
    df[:N_DISHES] = dish_feat

    def tile_featT(f, rows):
        nt = rows // P
        a = f[:rows].reshape(nt, P, 8, P)          # [r, m, k, p]
        a = a.transpose(3, 0, 2, 1).reshape(P, nt * 8 * P)  # [p, r, k, m]
        return np.ascontiguousarray(a).astype(ml_dtypes.bfloat16)

    wu_cat = np.concatenate([inputs["W_ui"], inputs["W_dm"]], axis=1)  # [1024, 512]
    wd_cat = np.concatenate([inputs["W_di"], inputs["W_um"]], axis=1)
    b_u = np.concatenate([inputs["b_ui"], inputs["b_dm"]])[None, :]
    b_d = np.concatenate([inputs["b_di"], inputs["b_um"]])[None, :]

    def reshape_w_proj(w):
        return np.ascontiguousarray(
            w.reshape(8, P, 512).transpose(1, 0, 2).reshape(P, 8 * 512)
        ).astype(ml_dtypes.bfloat16)

    def reshape_w_upd(w):  # [256,256] -> [128, 4*128]; (kk,mm) chunk at (kk*2+mm)*128
        return np.ascontiguousarray(
            w.reshape(2, P, 2, P).transpose(1, 0, 2, 3).reshape(P, 4 * P)
        ).astype(ml_dtypes.bfloat16)

    def reshape_w_out(w):  # [256,128] -> [128, 2*128]
        return np.ascontiguousarray(
            w.reshape(2, P, P).transpose(1, 0, 2).reshape(P, 2 * P)
        ).astype(ml_dtypes.bfloat16)

    biases = {
        "b_u": b_u, "b_d": b_d,
        "b_uu": inputs["b_uu"][None, :], "b_du": inputs["b_du"][None, :],
        "b_up": inputs["b_up"][None, :], "b_dp": inputs["b_dp"][None, :],
    }
    bias_zero = {k: bool(np.all(np.asarray(v) == 0)) for k, v in biases.items()}

    consts = {
        "wu": reshape_w_proj(wu_cat),
        "wd": reshape_w_proj(wd_cat),
        "wuu": reshape_w_upd(inputs["W_uu"]),
        "wdu": reshape_w_upd(inputs["W_du"]),
        "wup": reshape_w_out(inputs["W_up"]),
        "wdp": reshape_w_out(inputs["W_dp"]),
    }
    consts.update({k: np.asarray(v).astype(ml_dtypes.bfloat16) for k, v in biases.items()})

    ublock = edge_user // P
    uowner = ublock // UBC
    downer = edge_user // U_ROWS

    Tu_b, user_side = _build_side(
        recv_all=edge_user, send_all=edge_dish, owner_of_edge=uowner,
        n_blocks=UBC, block_fn=lambda r, c: (r // P) - c * UBC,
        send_base=[0] * NCORES,
    )
    # dish blocks in rs-chunk-major order so each RS chunk's partials finish
    # in one contiguous run of device steps
    GRP = DBC // RS_CHUNKS
    dish_perm = np.array([c * DBC + j * GRP + k
                          for j in range(RS_CHUNKS)
                          for c in range(NCORES)
                          for k in range(GRP)])
    Td_b, dish_side = _build_side(
        recv_all=edge_dish, send_all=edge_user, owner_of_edge=downer,
        n_blocks=DB, block_fn=lambda r, c: r // P,
        send_base=[c * U_ROWS for c in range(NCORES)],
        block_perm=dish_perm,
    )

    in_maps = []
    for c in range(NCORES):
        im = {
            "featTu": tile_featT(uf[c * U_ROWS:(c + 1) * U_ROWS], U_ROWS),
            "featTd": tile_featT(df[c * D_ROWS:(c + 1) * D_ROWS], D_ROWS),
            "uidx": user_side[c][0],
            "s_u": user_side[c][1],
            "didx": dish_side[c][0],
            "s_d": dish_side[c][1],
        }
        im.update(consts)
        in_maps.append(im)
    meta = (tuple(int(t) for t in Tu_b), tuple(int(t) for t in Td_b),
            tuple(int(b) for b in dish_perm), tuple(sorted(bias_zero.items())))
    return in_maps, meta


# --------------------------------------------------------------------------
# device kernel
# --------------------------------------------------------------------------

def _build(meta):
    Tu_b, Td_b, dish_perm, bias_zero_items = meta
    bias_zero = dict(bias_zero_items)
    offs_u = np.concatenate([[0], np.cumsum(Tu_b)]).astype(int)
    offs_d = np.concatenate([[0], np.cumsum(Td_b)]).astype(int)
    TOT_U = int(offs_u[-1])
    TOT_D = int(offs_d[-1])
    TMAX_U = int(max(Tu_b))
    TMAX_D = int(max(Td_b))
    TMAX = max(TMAX_U, TMAX_D)

    nc = bacc.Bacc("TRN2", debug=False, num_swdge_queues=NQ)

    featTu = nc.declare_dram_parameter("featTu", [P, UBC * 8 * P], BF16, isOutput=False)
    featTd = nc.declare_dram_parameter("featTd", [P, DBC * 8 * P], BF16, isOutput=False)
    wu = nc.declare_dram_parameter("wu", [P, 8 * 512], BF16, isOutput=False)
    wd = nc.declare_dram_parameter("wd", [P, 8 * 512], BF16, isOutput=False)
    wuu = nc.declare_dram_parameter("wuu", [P, 4 * P], BF16, isOutput=False)
    wdu = nc.declare_dram_parameter("wdu", [P, 4 * P], BF16, isOutput=False)
    wup = nc.declare_dram_parameter("wup", [P, 2 * P], BF16, isOutput=False)
    wdp = nc.declare_dram_parameter("wdp", [P, 2 * P], BF16, isOutput=False)
    b_u = nc.declare_dram_parameter("b_u", [1, 512], BF16, isOutput=False)
    b_d = nc.declare_dram_parameter("b_d", [1, 512], BF16, isOutput=False)
    b_uu = nc.declare_dram_parameter("b_uu", [1, HID], BF16, isOutput=False)
    b_du = nc.declare_dram_parameter("b_du", [1, HID], BF16, isOutput=False)
    b_up = nc.declare_dram_parameter("b_up", [1, OUT], BF16, isOutput=False)
    b_dp = nc.declare_dram_parameter("b_dp", [1, OUT], BF16, isOutput=False)
    uidx = nc.declare_dram_parameter("uidx", [P, TOT_U * 8], I16, isOutput=False)
    s_u = nc.declare_dram_parameter("s_u", [P, TOT_U * P], BF16, isOutput=False)
    didx = nc.declare_dram_parameter("didx", [P, TOT_D * 8], I16, isOutput=False)
    s_d = nc.declare_dram_parameter("s_d", [P, TOT_D * P], BF16, isOutput=False)
    user_emb = nc.declare_dram_parameter("user_emb", [U_ROWS, OUT], F32, isOutput=True)
    dish_emb = nc.declare_dram_parameter("dish_emb", [D_ROWS, OUT], F32, isOutput=True)

    rg = [list(range(NCORES))]
    RSB = D_ROWS // RS_CHUNKS  # 640 rows per core per RS chunk
    GRP = DBC // RS_CHUNKS     # 5 blocks per core per RS chunk

    with tile.TileContext(nc) as tc:
        with (
            tc.tile_pool(name="const", bufs=1) as cpool,
            tc.tile_pool(name="resid", bufs=1) as rpool,
            tc.tile_pool(name="dram", bufs=1, space="DRAM") as dram,
        ):
            # ---- constants
            wu_t = cpool.tile([P, 8 * 512], BF16)
            wd_t = cpool.tile([P, 8 * 512], BF16)
            wuu_t = cpool.tile([P, 4 * P], BF16)
            wdu_t = cpool.tile([P, 4 * P], BF16)
            wup_t = cpool.tile([P, 2 * P], BF16)
            wdp_t = cpool.tile([P, 2 * P], BF16)
            bias_t = cpool.tile([1, 512 * 2 + HID * 2 + OUT * 2], BF16)
            ones_t = cpool.tile([1, P], BF16)
            ident = cpool.tile([P, P], BF16)
            uidx_t = cpool.tile([P, TOT_U * 8], I16)
            didx_t = cpool.tile([P, TOT_D * 8], I16)
            for dst, src in [(wu_t, wu), (wd_t, wd), (wuu_t, wuu), (wdu_t, wdu),
                             (wup_t, wup), (wdp_t, wdp), (uidx_t, uidx), (didx_t, didx)]:
                nc.sync.dma_start(dst[:], src[:])
            off = 0
            bslc = {}
            for nm, src, w in [("b_u", b_u, 512), ("b_d", b_d, 512), ("b_uu", b_uu, HID),
                               ("b_du", b_du, HID), ("b_up", b_up, OUT), ("b_dp", b_dp, OUT)]:
                nc.sync.dma_start(bias_t[:1, off:off + w], src[:])
                bslc[nm] = (off, w)
                off += w
            nc.vector.memset(ones_t[:], 1.0)
            from concourse.masks import make_identity
            make_identity(nc, ident[:])

            user_init = rpool.tile([P, UBC * HID], BF16)
            dish_init = rpool.tile([P, DBC * HID], BF16)

            user_msg = dram.tile([U_ROWS, HID], BF16)
            ag_in = dram.tile([D_ROWS, HID], BF16)
            ag_out = dram.tile([D_PAD, HID], BF16)
            rs_in = [dram.tile([NCORES * RSB, HID], BF16, name=f"rs_in{j}") for j in range(RS_CHUNKS)]
            rs_out = [dram.tile([RSB, HID], BF16, name=f"rs_out{j}") for j in range(RS_CHUNKS)]

            def bias_mm(ps, key, ncols, m_off=0, bias_as_lhs=False):
                o, w = bslc[key]
                if bias_as_lhs:
                    nc.tensor.matmul(out=ps, lhsT=bias_t[:1, o + m_off:o + m_off + ncols],
                                     rhs=ones_t[:1, :], start=False, stop=True)
                else:
                    nc.tensor.matmul(out=ps, lhsT=ones_t[:1, :],
                                     rhs=bias_t[:1, o + m_off:o + m_off + ncols],
                                     start=False, stop=True)

            # ---- projection phases
            fpool_cm = tc.tile_pool(name="feat", bufs=4)
            fpool = fpool_cm.__enter__()
            mpool_cm = tc.tile_pool(name="msg", bufs=3)
            mpool = mpool_cm.__enter__()
            proj_ps_cm = tc.tile_pool(name="projps", bufs=2, space="PSUM")
            proj_ps = proj_ps_cm.__enter__()

            def proj_tile(r, featT, w_t, bias_key, init_tile, init_col, msg_dst_rows):
                ft = fpool.tile([P, 8 * P], BF16, tag="ft")
                nc.sync.dma_start(ft[:], featT[:, r * 8 * P:(r + 1) * 8 * P])
                ps = proj_ps.tile([P, 512], F32, tag="pps")
                zb = bias_zero[bias_key]
                for k in range(8):
                    nc.tensor.matmul(
                        out=ps[:], lhsT=ft[:, k * P:(k + 1) * P],
                        rhs=w_t[:, k * 512:(k + 1) * 512],
                        start=(k == 0), stop=(k == 7 and zb),
                    )
                if not zb:
                    bias_mm(ps[:], bias_key, 512)
                nc.scalar.activation(init_tile[:, init_col:init_col + HID], ps[:, :HID],
                                     mybir.ActivationFunctionType.Relu)
                mt = mpool.tile([P, HID], BF16, tag="mt")
                nc.scalar.activation(mt[:], ps[:, HID:], mybir.ActivationFunctionType.Relu)
                nc.sync.dma_start(msg_dst_rows, mt[:])

            def proj_half(r, featT, w_t, bias_key, half, dst_sbuf=None, dst_rows=None, ps_pool=None):
                """half=1: msg half (cols 256:512) -> dst_rows; half=0: init half."""
                ft = fpool.tile([P, 8 * P], BF16, tag="ft")
                nc.sync.dma_start(ft[:], featT[:, r * 8 * P:(r + 1) * 8 * P])
                pool = ps_pool or proj_ps
                ps = pool.tile([P, HID], F32, tag="aps" if ps_pool else "hps")
                zb = bias_zero[bias_key]
                for k in range(8):
                    nc.tensor.matmul(
                        out=ps[:], lhsT=ft[:, k * P:(k + 1) * P],
                        rhs=w_t[:, k * 512 + half * HID:k * 512 + (half + 1) * HID],
                        start=(k == 0), stop=(k == 7 and zb),
                    )
                if not zb:
                    bias_mm(ps[:], bias_key, HID, m_off=half * HID)
                if dst_sbuf is not None:
                    nc.scalar.activation(dst_sbuf, ps[:], mybir.ActivationFunctionType.Relu)
                else:
                    mt = mpool.tile([P, HID], BF16, tag="mt")
                    nc.scalar.activation(mt[:], ps[:], mybir.ActivationFunctionType.Relu)
                    nc.sync.dma_start(dst_rows, mt[:])

            # dish projection first -> AllGather fires early
            for r in range(DBC):
                proj_tile(r, featTd, wd_t, "b_d", dish_init, r * HID,
                          ag_in[r * P:(r + 1) * P, :])
            nc.gpsimd.collective_compute(
                "AllGather", mybir.AluOpType.bypass, replica_groups=rg,
                ins=[ag_in[:].opt()], outs=[ag_out[:].opt()],
            )
            # user projection: both halves in one pass (msg -> HBM, init -> SBUF)
            for r in range(UBC):
                proj_tile(r, featTu, wu_t, "b_u", user_init, r * HID,
                          user_msg[r * P:(r + 1) * P, :])
            proj_ps_cm.__exit__(None, None, None)
            mpool_cm.__exit__(None, None, None)
            fpool_cm.__exit__(None, None, None)

            # ---- interleaved aggregation/update phases
            with (
                tc.tile_pool(name="gath", bufs=8) as gpool,
                tc.tile_pool(name="spool", bufs=8) as spool,
                tc.tile_pool(name="aggps", bufs=4, space="PSUM") as aggps,
                tc.tile_pool(name="trps", bufs=1, space="PSUM") as trps,
                tc.tile_pool(name="pa", bufs=4) as papool,
                tc.tile_pool(name="z", bufs=3) as zpool,
                tc.tile_pool(name="zt", bufs=3) as ztpool,
                tc.tile_pool(name="updps", bufs=2, space="PSUM") as updps,
                tc.tile_pool(name="ut", bufs=3) as utpool,
                tc.tile_pool(name="embps", bufs=1, space="PSUM") as embps,
                tc.tile_pool(name="e", bufs=3) as epool,
            ):
                gq = [0]

                def agg_pair(table, idx_t, s_tab, offs, b0, nblk, tmax2):
                    """One gather call covering blocks b0..b0+nblk; returns list of
                    per-block psum [P, HID] f32."""
                    o0, o1 = int(offs[b0]), int(offs[b0 + nblk])
                    T = o1 - o0
                    g = gpool.tile([P, T, HID], BF16, tag="g", padded_shape=[P, tmax2, HID])
                    nc.gpsimd.dma_gather(
                        g[:], table[:], idx_t[:, o0 * 8:o1 * 8],
                        T * P, T * P, HID, single_packet=False, queue_num=gq[0] % NQ,
                    )
                    gq[0] += 1
                    st = spool.tile([P, T * P], BF16, tag="st", padded_shape=[P, tmax2 * P])
                    nc.sync.dma_start(st[:], s_tab[:, o0 * P:o1 * P])
                    res = []
                    for bb in range(nblk):
                        t0 = int(offs[b0 + bb]) - o0
                        t1 = int(offs[b0 + bb + 1]) - o0
                        ps = aggps.tile([P, HID], F32, tag="aps")
                        for t in range(t0, t1):
                            nc.tensor.matmul(out=ps[:], lhsT=st[:, t * P:(t + 1) * P],
                                             rhs=g[:, t, :], start=(t == t0), stop=(t == t1 - 1))
                        res.append(ps)
                    return res

                def update_block(z, w_upd, b_upd_key, w_out, b_out_key, emb_dst):
                    # z: [128, 256] bf16. PE transpose -> z_T chunks
                    pst = trps.tile([P, HID], BF16, tag="trp")
                    for j in range(2):
                        nc.tensor.transpose(pst[:, j * P:(j + 1) * P],
                                            z[:, j * P:(j + 1) * P], ident[:])
                    zt = ztpool.tile([P, HID], BF16, tag="zt")
                    nc.scalar.activation(zt[:], pst[:], mybir.ActivationFunctionType.Copy)
                    psu = updps.tile([P, HID], F32, tag="updp")
                    zb = bias_zero[b_upd_key]
                    for mm in range(2):
                        for kk in range(2):
                            nc.tensor.matmul(
                                out=psu[:, mm * P:(mm + 1) * P],
                                lhsT=w_upd[:, (kk * 2 + mm) * P:(kk * 2 + mm + 1) * P],
                                rhs=zt[:, kk * P:(kk + 1) * P],
                                start=(kk == 0), stop=(kk == 1 and zb),
                            )
                        if not zb:
                            bias_mm(psu[:, mm * P:(mm + 1) * P], b_upd_key, P,
                                    m_off=mm * P, bias_as_lhs=True)
                    ut = utpool.tile([P, HID], BF16, tag="ut")
                    nc.scalar.activation(ut[:], psu[:], mybir.ActivationFunctionType.Relu)
                    pse = embps.tile([P, OUT], F32, tag="embp")
                    zb2 = bias_zero[b_out_key]
                    for kk in range(2):
                        nc.tensor.matmul(out=pse[:], lhsT=ut[:, kk * P:(kk + 1) * P],
                                         rhs=w_out[:, kk * P:(kk + 1) * P],
                                         start=(kk == 0), stop=(kk == 1 and zb2))
                    if not zb2:
                        bias_mm(pse[:], b_out_key, OUT)
                    e = epool.tile([P, OUT], F32, tag="e")
                    nc.vector.tensor_copy(out=e[:], in_=pse[:])
                    nc.sync.dma_start(emb_dst, e[:])

                GCAP = 8   # max tiles (=128-idx groups) per gather call

                def make_groups(T_b, lo, hi):
                    groups = []
                    b = lo
                    while b < hi:
                        nblk, tot = 1, T_b[b]
                        while b + nblk < hi and tot + T_b[b + nblk] <= GCAP:
                            tot += T_b[b + nblk]
                            nblk += 1
                        groups.append((b, nblk))
                        b += nblk
                    return groups

                groups_d = []
                seg = DB // RS_CHUNKS
                for j in range(RS_CHUNKS):
                    groups_d += make_groups(Td_b, j * seg, (j + 1) * seg)
                groups_u = make_groups(Tu_b, 0, UBC)
                TM2_D = int(max(sum(Td_b[b0:b0 + nb]) for b0, nb in groups_d))
                TM2_U = int(max(sum(Tu_b[b0:b0 + nb]) for b0, nb in groups_u))

                def dish_scatter_pair(i0, nblk):
                    pss = agg_pair(user_msg, didx_t, s_d, offs_d, i0, nblk, TM2_D)
                    for bb, ps in enumerate(pss):
                        b = dish_perm[i0 + bb]     # original block id
                        pa = papool.tile([P, HID], BF16, tag="pa")
                        nc.vector.tensor_copy(out=pa[:], in_=ps[:])
                        c_own, r_own = b // DBC, b % DBC
                        j = r_own // GRP
                        row0 = c_own * RSB + (r_own % GRP) * P
                        nc.sync.dma_start(rs_in[j][row0:row0 + P, :], pa[:])

                def user_pair(b0, nblk):
                    pss = agg_pair(ag_out, uidx_t, s_u, offs_u, b0, nblk, TM2_U)
                    for bb, ps in enumerate(pss):
                        b = b0 + bb
                        z = zpool.tile([P, HID], BF16, tag="z")
                        nc.vector.tensor_tensor(out=z[:], in0=ps[:],
                                                in1=user_init[:, b * HID:(b + 1) * HID],
                                                op=mybir.AluOpType.add)
                        update_block(z, wuu_t, "b_uu", wup_t, "b_up",
                                     user_emb[b * P:(b + 1) * P, :])

                # user groups first (their gathers only wait on the AllGather),
                # keeping a small reserve to cover the last ReduceScatter; then
                # dish groups with RS chunks inline.
                RESERVE = 5
                for b0, nb in groups_u[:-RESERVE]:
                    user_pair(b0, nb)
                for i, (b0, nb) in enumerate(groups_d):
                    dish_scatter_pair(b0, nb)
                    done_blocks = b0 + nb
                    if done_blocks % (DB // RS_CHUNKS) == 0:
                        jj = done_blocks // (DB // RS_CHUNKS) - 1
                        nc.gpsimd.collective_compute(
                            "ReduceScatter", mybir.AluOpType.add, replica_groups=rg,
                            ins=[rs_in[jj][:].opt()], outs=[rs_out[jj][:].opt()],
                        )
                for b0, nb in groups_u[-RESERVE:]:
                    user_pair(b0, nb)

                # ---- dish update (needs RS results)
                for k in range(DBC):
                    j = k // GRP
                    row0 = (k % GRP) * P
                    at = zpool.tile([P, HID], BF16, tag="at")
                    nc.sync.dma_start(at[:], rs_out[j][row0:row0 + P, :])
                    z = zpool.tile([P, HID], BF16, tag="z")
                    nc.vector.tensor_tensor(out=z[:], in0=at[:],
                                            in1=dish_init[:, k * HID:(k + 1) * HID],
                                            op=mybir.AluOpType.add)
                    update_block(z, wdu_t, "b_du", wdp_t, "b_dp",
                                 dish_emb[k * P:(k + 1) * P, :])

    nc.compile()
    return nc


# --------------------------------------------------------------------------
# entry point
# --------------------------------------------------------------------------

def kernel(**inputs):
    in_maps, meta = _prep(inputs)
    if meta not in _cache:
        _cache[meta] = _build(meta)
    nc = _cache[meta]
    res = run_bass_kernel_spmd(nc, in_maps, core_ids=list(range(NCORES)))
    user_emb = np.concatenate([res.results[c]["user_emb"] for c in range(NCORES)], axis=0)[:N_USERS]
    dish_emb = np.concatenate([res.results[c]["dish_emb"] for c in range(NCORES)], axis=0)[:N_DISHES]
    return (user_emb.astype(np.float32), dish_emb.astype(np.float32))


# revision 20
# speedup vs baseline: 1.2595x; 1.2595x over previous
"""Bipartite GNN message-passing kernel for 8 Trainium2 NeuronCores.

Strategy (self-contained; shapes hardcoded for the nn_BipartiteGNN problem):

  reference computes   relu(feat[idx] @ W)  per edge. Row-gather commutes with
  the matmul, so we project FIRST (dense, small) and gather 256-dim message
  rows instead of 1024-dim feature rows:

      u_msg = relu(dish_feat @ W_um)[edge_dish]     (same for d_msg)

  Sharding over 8 cores:
    * users:  50176 padded rows -> 392 blocks of 128 -> 49 blocks/core.
      Core c projects its user rows (user_init kept in SBUF, user_msg -> HBM)
      and aggregates/updates its own user blocks.
    * dishes: 10240 padded rows -> 80 blocks -> 10 blocks/core for
      projection/update.  dish_msg slices are AllGathered (5.2 MB bf16) so
      every core can gather any dish row locally.
    * dish-side aggregation is sender-sharded: core c scatter-adds messages of
      edges whose sender user lives on c into a full [10240,256] partial,
      then two aligned ReduceScatter chunks hand each core the summed
      aggregates for its own dish blocks.

  segment_sum is done as one-hot matmuls: per receiver block, gather the
  edge message rows (dma_gather, rows land one-per-partition) and multiply by
  host-built one-hot S tiles, accumulating in PSUM.  Dummy slots gather row 0
  (the negative-index trim path crashes this runtime) and carry S=0 columns;
  per-block tile counts are the max over the 8 cores, so the static SPMD
  program stays identical while padding stays small.

  The dish-scatter and user-block phases are interleaved so the TensorEngine
  always has dense matmul work while gathers stream in the background (HAM
  stays at 2.4 GHz).
"""

import os
import sys

import numpy as np
import ml_dtypes

if "/opt/trn_rl_repo" not in sys.path:
    sys.path.insert(0, "/opt/trn_rl_repo")

import concourse.bass as bass  # noqa: E402
import concourse.mybir as mybir  # noqa: E402
import concourse.tile as tile  # noqa: E402
from concourse import bacc  # noqa: E402
from concourse.bass_utils import run_bass_kernel_spmd  # noqa: E402

P = 128
NCORES = 8
N_USERS = 50000
N_DISHES = 10000
D_IN = 1024
HID = 256
OUT = 128

U_PAD = 50176            # 392 blocks of 128
D_PAD = 10240            # 80 blocks of 128
UBC = U_PAD // P // NCORES   # 49 user blocks per core
DBC = D_PAD // P // NCORES   # 10 dish blocks per core (update phase)
DB = D_PAD // P              # 80 dish blocks (scatter phase, all cores)
U_ROWS = UBC * P             # 6272 user rows per core
D_ROWS = DBC * P             # 1280 dish rows per core
RS_CHUNKS = 2                # dish_agg ReduceScatter split (640 rows/core each)

BF16 = mybir.dt.bfloat16
F32 = mybir.dt.float32
I16 = mybir.dt.int16

_cache = {}
NQ = int(os.environ.get("KNQ", "3"))


# --------------------------------------------------------------------------
# host-side prep
# --------------------------------------------------------------------------

def _wrap_idx_calls(idx_flat: np.ndarray) -> np.ndarray:
    """[n] int16 -> [128, n/16] wrapped gather-index layout: logical index i at
    partition i%16 (replicated across the 8 groups of 16), free slot i//16."""
    n = idx_flat.shape[0]
    assert n % 16 == 0
    w = idx_flat.reshape(n // 16, 16).T  # [16, n/16]
    return np.tile(w, (8, 1)).astype(np.int16)


def _build_side(recv_all, send_all, owner_of_edge, n_blocks, block_fn, send_base,
                block_perm=None):
    """For one aggregation side, build per-core gather-index + one-hot-S data
    plus the shared per-block tile counts T_b (max over cores, >=1).

    block_perm[i] = original block id processed at device step i; the idx/S
    layout follows the permuted order so device tile ranges stay contiguous.
    Returns (T_b [n_blocks] in permuted order, per_core list of
    (idx_wrapped, s_tiles)).
    """
    if block_perm is None:
        block_perm = np.arange(n_blocks)
    inv_perm = np.empty(n_blocks, np.int64)
    inv_perm[np.asarray(block_perm)] = np.arange(n_blocks)
    per_core = []
    counts_all = np.zeros((NCORES, n_blocks), np.int64)
    for c in range(NCORES):
        m = owner_of_edge == c
        send = send_all[m] - send_base[c]
        blk = inv_perm[block_fn(recv_all[m], c)]   # permuted block index
        order = np.argsort(blk, kind="stable")
        rb = blk[order]
        rl = (recv_all[m][order] % P).astype(np.int64)
        sl = send[order].astype(np.int64)
        counts = np.bincount(rb, minlength=n_blocks)
        counts_all[c] = counts
        per_core.append((counts, rb, rl, sl))
    T_b = np.maximum(np.ceil(counts_all.max(axis=0) / P).astype(np.int64), 1)
    offs = np.concatenate([[0], np.cumsum(T_b)])
    total_tiles = int(offs[-1])

    out = []
    for c in range(NCORES):
        counts, rb, rl, sl = per_core[c]
        starts = np.concatenate([[0], np.cumsum(counts)[:-1]])
        within = np.arange(len(rb), dtype=np.int64) - starts[rb]
        idx_flat = np.zeros((total_tiles * P,), dtype=np.int16)
        slot = offs[rb] * P + within
        idx_flat[slot] = sl.astype(np.int16)
        s = np.zeros((P, total_tiles * P), dtype=ml_dtypes.bfloat16)
        s[slot % P, (offs[rb] + within // P) * P + rl] = 1.0
        out.append((_wrap_idx_calls(idx_flat), s))
    return T_b, out


def _prep(inputs):
    """All numpy preprocessing; returns (in_maps, meta)."""
    user_feat = inputs["user_feat"]
    dish_feat = inputs["dish_feat"]
    edge_user = np.asarray(inputs["edge_user"], dtype=np.int64)
    edge_dish = np.asarray(inputs["edge_dish"], dtype=np.int64)

    uf = np.zeros((U_PAD, D_IN), np.float32)
    uf[:N_USERS] = user_feat
    df = np.zeros((D_PAD, D_IN), np.float32)
    df[:N_DISHES] = dish_feat

    def tile_featT(f, rows):
        nt = rows // P
        a = f[:rows].reshape(nt, P, 8, P)          # [r, m, k, p]
        a = a.transpose(3, 0, 2, 1).reshape(P, nt * 8 * P)  # [p, r, k, m]
        return np.ascontiguousarray(a).astype(ml_dtypes.bfloat16)

    wu_cat = np.concatenate([inputs["W_ui"], inputs["W_dm"]], axis=1)  # [1024, 512]
    wd_cat = np.concatenate([inputs["W_di"], inputs["W_um"]], axis=1)
    b_u = np.concatenate([inputs["b_ui"], inputs["b_dm"]])[None, :]
    b_d = np.concatenate([inputs["b_di"], inputs["b_um"]])[None, :]

    def reshape_w_proj(w):
        return np.ascontiguousarray(
            w.reshape(8, P, 512).transpose(1, 0, 2).reshape(P, 8 * 512)
        ).astype(ml_dtypes.bfloat16)

    def reshape_w_upd(w):  # [256,256] -> [128, 4*128]; (kk,mm) chunk at (kk*2+mm)*128
        return np.ascontiguousarray(
            w.reshape(2, P, 2, P).transpose(1, 0, 2, 3).reshape(P, 4 * P)
        ).astype(ml_dtypes.bfloat16)

    def reshape_w_out(w):  # [256,128] -> [128, 2*128]
        return np.ascontiguousarray(
            w.reshape(2, P, P).transpose(1, 0, 2).reshape(P, 2 * P)
        ).astype(ml_dtypes.bfloat16)

    biases = {
        "b_u": b_u, "b_d": b_d,
        "b_uu": inputs["b_uu"][None, :], "b_du": inputs["b_du"][None, :],
        "b_up": inputs["b_up"][None, :], "b_dp": inputs["b_dp"][None, :],
    }
    bias_zero = {k: bool(np.all(np.asarray(v) == 0)) for k, v in biases.items()}

    consts = {
        "wu": reshape_w_proj(wu_cat),
        "wd": reshape_w_proj(wd_cat),
        "wuu": reshape_w_upd(inputs["W_uu"]),
        "wdu": reshape_w_upd(inputs["W_du"]),
        "wup": reshape_w_out(inputs["W_up"]),
        "wdp": reshape_w_out(inputs["W_dp"]),
    }
    consts.update({k: np.asarray(v).astype(ml_dtypes.bfloat16) for k, v in biases.items()})

    ublock = edge_user // P
    uowner = ublock // UBC
    downer = edge_user // U_ROWS

    Tu_b, user_side = _build_side(
        recv_all=edge_user, send_all=edge_dish, owner_of_edge=uowner,
        n_blocks=UBC, block_fn=lambda r, c: (r // P) - c * UBC,
        send_base=[0] * NCORES,
    )
    # dish blocks in rs-chunk-major order so each RS chunk's partials finish
    # in one contiguous run of device steps
    GRP = DBC // RS_CHUNKS
    dish_perm = np.array([c * DBC + j * GRP + k
                          for j in range(RS_CHUNKS)
                          for c in range(NCORES)
                          for k in range(GRP)])
    Td_b, dish_side = _build_side(
        recv_all=edge_dish, send_all=edge_user, owner_of_edge=downer,
        n_blocks=DB, block_fn=lambda r, c: r // P,
        send_base=[c * U_ROWS for c in range(NCORES)],
        block_perm=dish_perm,
    )

    in_maps = []
    for c in range(NCORES):
        im = {
            "featTu": tile_featT(uf[c * U_ROWS:(c + 1) * U_ROWS], U_ROWS),
            "featTd": tile_featT(df[c * D_ROWS:(c + 1) * D_ROWS], D_ROWS),
            "uidx": user_side[c][0],
            "s_u": user_side[c][1],
            "didx": dish_side[c][0],
            "s_d": dish_side[c][1],
        }
        im.update(consts)
        in_maps.append(im)
    meta = (tuple(int(t) for t in Tu_b), tuple(int(t) for t in Td_b),
            tuple(int(b) for b in dish_perm), tuple(sorted(bias_zero.items())))
    return in_maps, meta


# --------------------------------------------------------------------------
# device kernel
# --------------------------------------------------------------------------

def _build(meta):
    Tu_b, Td_b, dish_perm, bias_zero_items = meta
    bias_zero = dict(bias_zero_items)
    offs_u = np.concatenate([[0], np.cumsum(Tu_b)]).astype(int)
    offs_d = np.concatenate([[0], np.cumsum(Td_b)]).astype(int)
    TOT_U = int(offs_u[-1])
    TOT_D = int(offs_d[-1])
    TMAX_U = int(max(Tu_b))
    TMAX_D = int(max(Td_b))
    TMAX = max(TMAX_U, TMAX_D)

    nc = bacc.Bacc("TRN2", debug=False, num_swdge_queues=NQ)

    featTu = nc.declare_dram_parameter("featTu", [P, UBC * 8 * P], BF16, isOutput=False)
    featTd = nc.declare_dram_parameter("featTd", [P, DBC * 8 * P], BF16, isOutput=False)
    wu = nc.declare_dram_parameter("wu", [P, 8 * 512], BF16, isOutput=False)
    wd = nc.declare_dram_parameter("wd", [P, 8 * 512], BF16, isOutput=False)
    wuu = nc.declare_dram_parameter("wuu", [P, 4 * P], BF16, isOutput=False)
    wdu = nc.declare_dram_parameter("wdu", [P, 4 * P], BF16, isOutput=False)
    wup = nc.declare_dram_parameter("wup", [P, 2 * P], BF16, isOutput=False)
    wdp = nc.declare_dram_parameter("wdp", [P, 2 * P], BF16, isOutput=False)
    b_u = nc.declare_dram_parameter("b_u", [1, 512], BF16, isOutput=False)
    b_d = nc.declare_dram_parameter("b_d", [1, 512], BF16, isOutput=False)
    b_uu = nc.declare_dram_parameter("b_uu", [1, HID], BF16, isOutput=False)
    b_du = nc.declare_dram_parameter("b_du", [1, HID], BF16, isOutput=False)
    b_up = nc.declare_dram_parameter("b_up", [1, OUT], BF16, isOutput=False)
    b_dp = nc.declare_dram_parameter("b_dp", [1, OUT], BF16, isOutput=False)
    uidx = nc.declare_dram_parameter("uidx", [P, TOT_U * 8], I16, isOutput=False)
    s_u = nc.declare_dram_parameter("s_u", [P, TOT_U * P], BF16, isOutput=False)
    didx = nc.declare_dram_parameter("didx", [P, TOT_D * 8], I16, isOutput=False)
    s_d = nc.declare_dram_parameter("s_d", [P, TOT_D * P], BF16, isOutput=False)
    user_emb = nc.declare_dram_parameter("user_emb", [U_ROWS, OUT], F32, isOutput=True)
    dish_emb = nc.declare_dram_parameter("dish_emb", [D_ROWS, OUT], F32, isOutput=True)

    rg = [list(range(NCORES))]
    RSB = D_ROWS // RS_CHUNKS  # 640 rows per core per RS chunk
    GRP = DBC // RS_CHUNKS     # 5 blocks per core per RS chunk

    with tile.TileContext(nc) as tc:
        with (
            tc.tile_pool(name="const", bufs=1) as cpool,
            tc.tile_pool(name="resid", bufs=1) as rpool,
            tc.tile_pool(name="dram", bufs=1, space="DRAM") as dram,
        ):
            # ---- constants
            wu_t = cpool.tile([P, 8 * 512], BF16)
            wd_t = cpool.tile([P, 8 * 512], BF16)
            wuu_t = cpool.tile([P, 4 * P], BF16)
            wdu_t = cpool.tile([P, 4 * P], BF16)
            wup_t = cpool.tile([P, 2 * P], BF16)
            wdp_t = cpool.tile([P, 2 * P], BF16)
            bias_t = cpool.tile([1, 512 * 2 + HID * 2 + OUT * 2], BF16)
            ones_t = cpool.tile([1, P], BF16)
            ident = cpool.tile([P, P], BF16)
            uidx_t = cpool.tile([P, TOT_U * 8], I16)
            didx_t = cpool.tile([P, TOT_D * 8], I16)
            for dst, src in [(wu_t, wu), (wd_t, wd), (wuu_t, wuu), (wdu_t, wdu),
                             (wup_t, wup), (wdp_t, wdp), (uidx_t, uidx), (didx_t, didx)]:
                nc.sync.dma_start(dst[:], src[:])
            off = 0
            bslc = {}
            for nm, src, w in [("b_u", b_u, 512), ("b_d", b_d, 512), ("b_uu", b_uu, HID),
                               ("b_du", b_du, HID), ("b_up", b_up, OUT), ("b_dp", b_dp, OUT)]:
                nc.sync.dma_start(bias_t[:1, off:off + w], src[:])
                bslc[nm] = (off, w)
                off += w
            nc.vector.memset(ones_t[:], 1.0)
            from concourse.masks import make_identity
            make_identity(nc, ident[:])

            user_init = rpool.tile([P, UBC * HID], BF16)
            dish_init = rpool.tile([P, DBC * HID], BF16)

            user_msg = dram.tile([U_ROWS, HID], BF16)
            ag_in = dram.tile([D_ROWS, HID], BF16)
            ag_out = dram.tile([D_PAD, HID], BF16)
            rs_in = [dram.tile([NCORES * RSB, HID], BF16, name=f"rs_in{j}") for j in range(RS_CHUNKS)]
            rs_out = [dram.tile([RSB, HID], BF16, name=f"rs_out{j}") for j in range(RS_CHUNKS)]

            def bias_mm(ps, key, ncols, m_off=0, bias_as_lhs=False):
                o, w = bslc[key]
                if bias_as_lhs:
                    nc.tensor.matmul(out=ps, lhsT=bias_t[:1, o + m_off:o + m_off + ncols],
                                     rhs=ones_t[:1, :], start=False, stop=True)
                else:
                    nc.tensor.matmul(out=ps, lhsT=ones_t[:1, :],
                                     rhs=bias_t[:1, o + m_off:o + m_off + ncols],
                                     start=False, stop=True)

            # ---- projection phases
            fpool_cm = tc.tile_pool(name="feat", bufs=4)
            fpool = fpool_cm.__enter__()
            mpool_cm = tc.tile_pool(name="msg", bufs=3)
            mpool = mpool_cm.__enter__()
            proj_ps_cm = tc.tile_pool(name="projps", bufs=2, space="PSUM")
            proj_ps = proj_ps_cm.__enter__()

            def proj_tile(r, featT, w_t, bias_key, init_tile, init_col, msg_dst_rows):
                ft = fpool.tile([P, 8 * P], BF16, tag="ft")
                nc.sync.dma_start(ft[:], featT[:, r * 8 * P:(r + 1) * 8 * P])
                ps = proj_ps.tile([P, 512], F32, tag="pps")
                zb = bias_zero[bias_key]
                for k in range(8):
                    nc.tensor.matmul(
                        out=ps[:], lhsT=ft[:, k * P:(k + 1) * P],
                        rhs=w_t[:, k * 512:(k + 1) * 512],
                        start=(k == 0), stop=(k == 7 and zb),
                    )
                if not zb:
                    bias_mm(ps[:], bias_key, 512)
                nc.scalar.activation(init_tile[:, init_col:init_col + HID], ps[:, :HID],
                                     mybir.ActivationFunctionType.Relu)
                mt = mpool.tile([P, HID], BF16, tag="mt")
                nc.scalar.activation(mt[:], ps[:, HID:], mybir.ActivationFunctionType.Relu)
                nc.sync.dma_start(msg_dst_rows, mt[:])

            def proj_half(r, featT, w_t, bias_key, half, dst_sbuf=None, dst_rows=None, ps_pool=None):
                """half=1: msg half (cols 256:512) -> dst_rows; half=0: init half."""
                ft = fpool.tile([P, 8 * P], BF16, tag="ft")
                nc.sync.dma_start(ft[:], featT[:, r * 8 * P:(r + 1) * 8 * P])
                pool = ps_pool or proj_ps
                ps = pool.tile([P, HID], F32, tag="aps" if ps_pool else "hps")
                zb = bias_zero[bias_key]
                for k in range(8):
                    nc.tensor.matmul(
                        out=ps[:], lhsT=ft[:, k * P:(k + 1) * P],
                        rhs=w_t[:, k * 512 + half * HID:k * 512 + (half + 1) * HID],
                        start=(k == 0), stop=(k == 7 and zb),
                    )
                if not zb:
                    bias_mm(ps[:], bias_key, HID, m_off=half * HID)
                if dst_sbuf is not None:
                    nc.scalar.activation(dst_sbuf, ps[:], mybir.ActivationFunctionType.Relu)
                else:
                    mt = mpool.tile([P, HID], BF16, tag="mt")
                    nc.scalar.activation(mt[:], ps[:], mybir.ActivationFunctionType.Relu)
                    nc.sync.dma_start(dst_rows, mt[:])

            # dish projection first -> AllGather fires early
            for r in range(DBC):
                proj_tile(r, featTd, wd_t, "b_d", dish_init, r * HID,
                          ag_in[r * P:(r + 1) * P, :])
            nc.gpsimd.collective_compute(
                "AllGather", mybir.AluOpType.bypass, replica_groups=rg,
                ins=[ag_in[:].opt()], outs=[ag_out[:].opt()],
            )
            # user projection: both halves in one pass (msg -> HBM, init -> SBUF)
            for r in range(UBC):
                proj_tile(r, featTu, wu_t, "b_u", user_init, r * HID,
                          user_msg[r * P:(r + 1) * P, :])
            # Only the PSUM pool is released; keeping the SBUF pools open
            # prevents the gather pools from reusing their space, which would
            # add false WAR edges delaying the first gathers ~60us.
            proj_ps_cm.__exit__(None, None, None)

            # ---- interleaved aggregation/update phases
            with (
                tc.tile_pool(name="gath", bufs=8) as gpool,
                tc.tile_pool(name="spool", bufs=8) as spool,
                tc.tile_pool(name="aggps", bufs=4, space="PSUM") as aggps,
                tc.tile_pool(name="trps", bufs=1, space="PSUM") as trps,
                tc.tile_pool(name="pa", bufs=4) as papool,
                tc.tile_pool(name="z", bufs=3) as zpool,
                tc.tile_pool(name="zt", bufs=3) as ztpool,
                tc.tile_pool(name="updps", bufs=2, space="PSUM") as updps,
                tc.tile_pool(name="ut", bufs=3) as utpool,
                tc.tile_pool(name="embps", bufs=1, space="PSUM") as embps,
                tc.tile_pool(name="e", bufs=3) as epool,
            ):
                gq = [0]

                def agg_pair(table, idx_t, s_tab, offs, b0, nblk, tmax2):
                    """One gather call covering blocks b0..b0+nblk; returns list of
                    per-block psum [P, HID] f32."""
                    o0, o1 = int(offs[b0]), int(offs[b0 + nblk])
                    T = o1 - o0
                    g = gpool.tile([P, T, HID], BF16, tag="g", padded_shape=[P, tmax2, HID])
                    nc.gpsimd.dma_gather(
                        g[:], table[:], idx_t[:, o0 * 8:o1 * 8],
                        T * P, T * P, HID, single_packet=False, queue_num=gq[0] % NQ,
                    )
                    gq[0] += 1
                    st = spool.tile([P, T * P], BF16, tag="st", padded_shape=[P, tmax2 * P])
                    nc.sync.dma_start(st[:], s_tab[:, o0 * P:o1 * P])
                    res = []
                    for bb in range(nblk):
                        t0 = int(offs[b0 + bb]) - o0
                        t1 = int(offs[b0 + bb + 1]) - o0
                        ps = aggps.tile([P, HID], F32, tag="aps")
                        for t in range(t0, t1):
                            nc.tensor.matmul(out=ps[:], lhsT=st[:, t * P:(t + 1) * P],
                                             rhs=g[:, t, :], start=(t == t0), stop=(t == t1 - 1))
                        res.append(ps)
                    return res

                def update_block(z, w_upd, b_upd_key, w_out, b_out_key, emb_dst):
                    # z: [128, 256] bf16. PE transpose -> z_T chunks
                    pst = trps.tile([P, HID], BF16, tag="trp")
                    for j in range(2):
                        nc.tensor.transpose(pst[:, j * P:(j + 1) * P],
                                            z[:, j * P:(j + 1) * P], ident[:])
                    zt = ztpool.tile([P, HID], BF16, tag="zt")
                    nc.scalar.activation(zt[:], pst[:], mybir.ActivationFunctionType.Copy)
                    psu = updps.tile([P, HID], F32, tag="updp")
                    zb = bias_zero[b_upd_key]
                    for mm in range(2):
                        for kk in range(2):
                            nc.tensor.matmul(
                                out=psu[:, mm * P:(mm + 1) * P],
                                lhsT=w_upd[:, (kk * 2 + mm) * P:(kk * 2 + mm + 1) * P],
                                rhs=zt[:, kk * P:(kk + 1) * P],
                                start=(kk == 0), stop=(kk == 1 and zb),
                            )
                        if not zb:
                            bias_mm(psu[:, mm * P:(mm + 1) * P], b_upd_key, P,
                                    m_off=mm * P, bias_as_lhs=True)
                    ut = utpool.tile([P, HID], BF16, tag="ut")
                    nc.scalar.activation(ut[:], psu[:], mybir.ActivationFunctionType.Relu)
                    pse = embps.tile([P, OUT], F32, tag="embp")
                    zb2 = bias_zero[b_out_key]
                    for kk in range(2):
                        nc.tensor.matmul(out=pse[:], lhsT=ut[:, kk * P:(kk + 1) * P],
                                         rhs=w_out[:, kk * P:(kk + 1) * P],
                                         start=(kk == 0), stop=(kk == 1 and zb2))
                    if not zb2:
                        bias_mm(pse[:], b_out_key, OUT)
                    e = epool.tile([P, OUT], F32, tag="e")
                    nc.vector.tensor_copy(out=e[:], in_=pse[:])
                    nc.sync.dma_start(emb_dst, e[:])

                GCAP = 8   # max tiles (=128-idx groups) per gather call

                def make_groups(T_b, lo, hi):
                    groups = []
                    b = lo
                    while b < hi:
                        nblk, tot = 1, T_b[b]
                        while b + nblk < hi and tot + T_b[b + nblk] <= GCAP:
                            tot += T_b[b + nblk]
                            nblk += 1
                        groups.append((b, nblk))
                        b += nblk
                    return groups

                groups_d = []
                seg = DB // RS_CHUNKS
                for j in range(RS_CHUNKS):
                    groups_d += make_groups(Td_b, j * seg, (j + 1) * seg)
                groups_u = make_groups(Tu_b, 0, UBC)
                TM2_D = int(max(sum(Td_b[b0:b0 + nb]) for b0, nb in groups_d))
                TM2_U = int(max(sum(Tu_b[b0:b0 + nb]) for b0, nb in groups_u))

                def dish_scatter_pair(i0, nblk):
                    pss = agg_pair(user_msg, didx_t, s_d, offs_d, i0, nblk, TM2_D)
                    for bb, ps in enumerate(pss):
                        b = dish_perm[i0 + bb]     # original block id
                        pa = papool.tile([P, HID], BF16, tag="pa")
                        nc.vector.tensor_copy(out=pa[:], in_=ps[:])
                        c_own, r_own = b // DBC, b % DBC
                        j = r_own // GRP
                        row0 = c_own * RSB + (r_own % GRP) * P
                        nc.sync.dma_start(rs_in[j][row0:row0 + P, :], pa[:])

                def user_pair(b0, nblk):
                    pss = agg_pair(ag_out, uidx_t, s_u, offs_u, b0, nblk, TM2_U)
                    for bb, ps in enumerate(pss):
                        b = b0 + bb
                        z = zpool.tile([P, HID], BF16, tag="z")
                        nc.vector.tensor_tensor(out=z[:], in0=ps[:],
                                                in1=user_init[:, b * HID:(b + 1) * HID],
                                                op=mybir.AluOpType.add)
                        update_block(z, wuu_t, "b_uu", wup_t, "b_up",
                                     user_emb[b * P:(b + 1) * P, :])

                # interleave user groups (gathers gated only by the AllGather)
                # with dish groups (gated by user_msg), keeping a reserve of
                # user groups to cover the last ReduceScatter.
                RESERVE = 4
                n_up_main = len(groups_u) - RESERVE
                n_dp = len(groups_d)
                ugi = 0
                for i, (b0, nb) in enumerate(groups_d):
                    while ugi < n_up_main and ugi * n_dp < (i + 1) * n_up_main:
                        user_pair(*groups_u[ugi])
                        ugi += 1
                    dish_scatter_pair(b0, nb)
                    done_blocks = b0 + nb
                    if done_blocks % (DB // RS_CHUNKS) == 0:
                        jj = done_blocks // (DB // RS_CHUNKS) - 1
                        nc.gpsimd.collective_compute(
                            "ReduceScatter", mybir.AluOpType.add, replica_groups=rg,
                            ins=[rs_in[jj][:].opt()], outs=[rs_out[jj][:].opt()],
                        )
                for b0, nb in groups_u[n_up_main:]:
                    user_pair(b0, nb)

                # ---- dish update (needs RS results)
                for k in range(DBC):
                    j = k // GRP
                    row0 = (k % GRP) * P
                    at = zpool.tile([P, HID], BF16, tag="at")
                    nc.sync.dma_start(at[:], rs_out[j][row0:row0 + P, :])
                    z = zpool.tile([P, HID], BF16, tag="z")
                    nc.vector.tensor_tensor(out=z[:], in0=at[:],
                                            in1=dish_init[:, k * HID:(k + 1) * HID],
                                            op=mybir.AluOpType.add)
                    update_block(z, wdu_t, "b_du", wdp_t, "b_dp",
                                 dish_emb[k * P:(k + 1) * P, :])

            mpool_cm.__exit__(None, None, None)
            fpool_cm.__exit__(None, None, None)

    nc.compile()
    return nc


# --------------------------------------------------------------------------
# entry point
# --------------------------------------------------------------------------

def kernel(**inputs):
    in_maps, meta = _prep(inputs)
    if meta not in _cache:
        _cache[meta] = _build(meta)
    nc = _cache[meta]
    res = run_bass_kernel_spmd(nc, in_maps, core_ids=list(range(NCORES)))
    user_emb = np.concatenate([res.results[c]["user_emb"] for c in range(NCORES)], axis=0)[:N_USERS]
    dish_emb = np.concatenate([res.results[c]["dish_emb"] for c in range(NCORES)], axis=0)[:N_DISHES]
    return (user_emb.astype(np.float32), dish_emb.astype(np.float32))
